# revision 26
# baseline (speedup 1.0000x reference)
"""Trainium2 Bass kernel for nn_Attention_1503238553757 (LSA attention) — v3.

Reference computation (per batch element):
    qkv = x @ w_qkv; q,k,v heads of dim 64
    dots = (q @ k^T) * scale[h]; diagonal masked to -inf
    attn = softmax(dots); out = attn @ v
    y = concat_heads(out) @ w_out + b_out

Sharding: data-parallel over batch (16 batches -> 2 per core x 8 cores).

v3 changes vs v2 (258us baseline):
  - PSUM at 1-bank granularity: scores are four [128,512] f32 tiles per
    round (per head x ih) in a shared 4-slot ring (psA) that filler
    evac tiles also ride; attnV accumulators are four [128,512] tiles
    (psO ring).  The v2 ring-of-2 shared by scores+fillers serialized
    the whole pipeline (measured 3.07us rounds, MM issue gaps ~290ns).
  - True PE row-group concurrency: score MMs alternate row groups
    (head-even at rows 0-63, head-odd at 64-127) with adjacent queue
    positions so pairs run concurrently; attnV is split into K=64
    j-halves that pair across different psum banks (never same-bank
    concurrent) for the same 2x effect.
  - attnV stationary padded to 128 cols (v | ones | zeros) -> FWL
    weight loads; psum rows 65-127 are garbage and never read.
  - exp ops are per-ih [128,512]: ACT true-exp for head-even, DVE
    Schraudolph for head-odd; psum slots free in ~600ns.
  - yproj evac uses scalar_tensor_tensor (psum + bias in one DVE op).
"""

import os
import sys

for _p in ("/opt/trn_rl_repo", "/root/.axon_site/_ro/trn_rl_repo"):
    if os.path.isdir(_p) and _p not in sys.path:
        sys.path.insert(0, _p)

import numpy as np

import concourse.bass as bass
import concourse.bacc as bacc
import concourse.tile as tile
import concourse.mybir as mybir
from concourse.bass_utils import run_bass_kernel_spmd

# Problem constants (hardcoded per harness contract)
B, N, D = 16, 1024, 512
HEADS, DH = 8, 64
N_CORES = 8
BPC = B // N_CORES  # batches per core = 2

dt = mybir.dt
F32 = dt.float32
BF16 = dt.bfloat16
F16 = dt.float16
I16 = dt.int16
EXP = mybir.ActivationFunctionType.Exp
MUL = mybir.AluOpType.mult
ADD = mybir.AluOpType.add

NT = N // 128   # token tiles = 8
KD = D // 128   # d/inner k-tiles = 4

# Schraudolph bf16-exp constants (DVE f32->int16 is round-to-nearest):
# exp(x) ~= bitcast_bf16(int16(A16*x + B16))
A16 = 128.0 / float(np.log(2.0))     # 184.6650
B16 = 127.0 * 128.0 - 7.4115         # 16248.59

# debug knobs for bisecting
V3_STT = os.environ.get("V3_STT", "0") == "1"
V3_SPLIT = os.environ.get("V3_SPLIT", "0") == "1"
V3_PIH = os.environ.get("V3_PIH", "1") == "1"  # per-ih scores/exp tiles
V3_PADV = os.environ.get("V3_PADV", "1") == "1"  # padded [*,128] v stationary
V3_PSO = os.environ.get("V3_PSO", "1") == "1"    # per-ih psO accumulators
VW = DH + 1


class EngBal:
    """Static load balancer between the Scalar (act) and Vector (dve)
    engines for PSUM-consuming ops."""

    def __init__(self, nc):
        self.nc = nc
        self.t = {"act": 0.0, "dve": 0.0}

    def pick(self, cost_act, cost_dve):
        if self.t["act"] + cost_act <= self.t["dve"] + cost_dve:
            self.t["act"] += cost_act
            return "act"
        self.t["dve"] += cost_dve
        return "dve"

    def add(self, eng, cost):
        self.t[eng] += cost


def build_program():
    nc = bacc.Bacc("TRN2", target_bir_lowering=False, debug=False,
                   num_devices=N_CORES)

    x = nc.dram_tensor("x", [BPC, N, D], F32, kind="ExternalInput").ap()
    w_qkv = nc.dram_tensor("w_qkv", [D, 3 * D], F32, kind="ExternalInput").ap()
    w_out = nc.dram_tensor("w_out", [D, D], F32, kind="ExternalInput").ap()
    b_out = nc.dram_tensor("b_out", [D], F32, kind="ExternalInput").ap()
    scale = nc.dram_tensor("scale", [HEADS], F32, kind="ExternalInput").ap()
    y = nc.dram_tensor("y", [BPC, N, D], F32, kind="ExternalOutput").ap()

    ident_dram = nc.inline_tensor(np.eye(128, dtype=np.float16), name="ident")
    ident32_dram = nc.inline_tensor(np.eye(128, dtype=np.float32),
                                    name="ident32")

    bal = EngBal(nc)

    import contextlib
    with tile.TileContext(nc) as tc, contextlib.ExitStack() as ctx:
        consts = ctx.enter_context(tc.tile_pool(name="consts", bufs=1))
        p_x = ctx.enter_context(tc.tile_pool(name="p_x", bufs=1))
        p_big = ctx.enter_context(tc.tile_pool(name="p_big", bufs=2))
        p_exp = ctx.enter_context(tc.tile_pool(name="p_exp", bufs=22))
        p_on = ctx.enter_context(tc.tile_pool(name="p_on", bufs=4))
        p_rb = ctx.enter_context(tc.tile_pool(name="p_rb", bufs=4))
        p_sm = ctx.enter_context(tc.tile_pool(name="p_sm", bufs=4))
        p_y = ctx.enter_context(tc.tile_pool(name="p_y", bufs=3))
        psA = ctx.enter_context(tc.tile_pool(name="psA", bufs=(4 if V3_PIH else 2),
                                             space="PSUM"))
        psO = ctx.enter_context(tc.tile_pool(name="psO", bufs=(4 if V3_PSO else 2),
                                             space="PSUM"))
        p_dram = ctx.enter_context(tc.tile_pool(name="p_dram", bufs=4,
                                                space="DRAM"))

        # ---------------- constants ----------------
        ident_sb = consts.tile([128, 128], F16)
        nc.sync.dma_start(out=ident_sb, in_=ident_dram.ap())
        inv_eye = consts.tile([128, 128], BF16)
        nc.vector.tensor_scalar(out=inv_eye, in0=ident_sb, scalar1=-1.0,
                                scalar2=1.0, op0=MUL, op1=ADD)
        wqkv_sb = consts.tile([128, KD, 3 * D], F16)
        wout_sb = consts.tile([128, KD, D], F16)
        bout_bc = consts.tile([128, D], F32)
        scale_sb = consts.tile([128, HEADS], F32)
        scale_schr = consts.tile([128, HEADS], F32)

        def emit_const_dmas_early():
            # q/k columns of w_qkv first (prologue critical path)
            nc.gpsimd.dma_start(
                out=wqkv_sb[:, :, 0:2 * D],
                in_=w_qkv.rearrange("(k p) c -> p k c", p=128)[:, :, 0:2 * D],
            )
            nc.gpsimd.dma_start(
                out=wqkv_sb[:, :, 2 * D:3 * D],
                in_=w_qkv.rearrange("(k p) c -> p k c", p=128)[:, :, 2 * D:3 * D],
            )
            nc.sync.dma_start(
                out=bout_bc,
                in_=bass.AP(tensor=b_out.tensor, offset=0,
                            ap=[[0, 128], [1, D]]),
            )
            nc.sync.dma_start(
                out=scale_sb,
                in_=bass.AP(tensor=scale.tensor, offset=0,
                            ap=[[0, 128], [1, HEADS]]),
            )
            nc.vector.tensor_scalar_mul(scale_schr, scale_sb, float(A16))

        def emit_const_dmas_late():
            # kt=3 halves swapped so pair g=3 packs (ho | he) in osb;
            # lets the tail normalize run shift-free on DVE for head 7.
            src = w_out.rearrange("(k p) c -> p k c", p=128)
            nc.gpsimd.dma_start(out=wout_sb[:, 0:KD - 1, :],
                                in_=src[:, 0:KD - 1, :])
            nc.gpsimd.dma_start(out=wout_sb[0:64, KD - 1, :],
                                in_=src[64:128, KD - 1, :])
            nc.gpsimd.dma_start(out=wout_sb[64:128, KD - 1, :],
                                in_=src[0:64, KD - 1, :])

        # ---------------- per-batch state ----------------
        xT = [None] * BPC
        qkT = [None] * BPC
        vsb = [None] * BPC   # [128, NT, HEADS, 128]: v | ones | zeros
        osb = [None] * BPC

        def alloc_batch(b):
            xT[b] = p_big.tile([128, KD, N], F16, tag="xT", name=f"xT{b}")
            qkT[b] = p_big.tile([128, 8, N], F16, tag="qk", name=f"qkT{b}")
            if V3_PADV:
                vsb[b] = p_big.tile([128, NT, HEADS, 128], BF16, tag="v",
                                    name=f"v{b}")
            else:
                vsb[b] = p_big.tile([128, NT, HEADS * VW + 64], BF16, tag="v",
                                    name=f"v{b}")
            osb[b] = p_big.tile([128, KD, N], F16, tag="o", name=f"o{b}")

        def emit_load_x(b, chunks=range(4), x_sb=None):
            # f32->f16 casting DMAs must go through the gpsimd queue; the
            # serial queue also keeps chunk 0 (prologue critical path)
            # ahead of the weight streams in DMA bandwidth.
            if x_sb is None:
                x_sb = p_x.tile([128, NT, D], F16, tag="x", name=f"x_sb{b}")
            src = x[b].rearrange("(r p) d -> p r d", p=128)
            for c in chunks:
                nc.gpsimd.dma_start(out=x_sb[:, 2 * c:2 * c + 2, :],
                                    in_=src[:, 2 * c:2 * c + 2, :])
            return x_sb

        def emit_vsb_init(b):
            if V3_PADV:
                # ones column at DH, zeros at DH+1.. (padded stationary cols)
                nc.gpsimd.memset(vsb[b][:, :, :, DH:DH + 1], 1.0)
                nc.gpsimd.memset(vsb[b][:, :, :, DH + 1:], 0.0)
            else:
                nc.gpsimd.memset(
                    vsb[b][:, :, 0:HEADS * VW].rearrange(
                        "p r (h e) -> p r h e", h=HEADS)[:, :, :, DH:DH + 1],
                    1.0,
                )
                nc.gpsimd.memset(vsb[b][:, :, HEADS * VW:], 1.0)

        # ---------------- filler units (ride the psA ring) ----------------
        def evac(dst_ap, src_ap, cost_scale=1.0, eng=None):
            # In-round filler evacs are pinned to ACT: its two exps finish
            # ~2.3us into the round, so the evac lands ~2.9us -- just in
            # time for the next round's 4th score MM, which recycles the
            # filler's psA slot (the 5-allocs-on-4-slots wraparound victim).
            e = eng or bal.pick(0.57 * cost_scale, 0.66 * cost_scale)
            if e == "act":
                bal.add("act", 0.57 * cost_scale) if eng else None
                nc.scalar.copy(dst_ap, src_ap)
            else:
                nc.vector.tensor_copy(dst_ap, src_ap)

        def u_tr(b, x_sb, kd, half):
            ps_t = psA.tile([128, 512], F16, tag="a",
                            name=f"ps_t{b}_{kd}_{half}")
            for rr in range(4):
                r = 4 * half + rr
                nc.tensor.transpose(
                    ps_t[:, 128 * rr:128 * rr + 128],
                    x_sb[:, r, 128 * kd:128 * kd + 128],
                    ident_sb,
                )
            evac(xT[b][:, kd, 512 * half:512 * half + 512], ps_t)

        def u_qk(b, ct, nh):
            ps_qk = psA.tile([128, 512], F32, tag="a",
                             name=f"ps_qk{b}_{ct}_{nh}")
            for kt in range(KD):
                nc.tensor.matmul(
                    ps_qk,
                    wqkv_sb[:, kt, 128 * ct:128 * ct + 128],
                    xT[b][:, kt, 512 * nh:512 * nh + 512],
                    start=(kt == 0), stop=(kt == KD - 1),
                )
            evac(qkT[b][:, ct, 512 * nh:512 * nh + 512], ps_qk)

        def u_v(b, r):
            ps_v = psA.tile([128, 512], F32, tag="a", name=f"ps_v{b}_{r}")
            for kt in range(KD):
                nc.tensor.matmul(
                    ps_v,
                    xT[b][:, kt, 128 * r:128 * r + 128],
                    wqkv_sb[:, kt, 2 * D:3 * D],
                    start=(kt == 0), stop=(kt == KD - 1),
                )
            if V3_PADV:
                evac(
                    vsb[b][:, r, :, 0:DH],
                    ps_v.rearrange("p (h e) -> p h e", h=HEADS),
                )
            else:
                evac(
                    vsb[b][:, r, 0:HEADS * VW].rearrange(
                        "p (h e) -> p h e", h=HEADS)[:, :, 0:DH],
                    ps_v.rearrange("p (h e) -> p h e", h=HEADS),
                )

        def u_yp(b, r, pool=None):
            pool = pool if pool is not None else psA
            tag = "a" if pool is psA else "o"
            ps_y = pool.tile([128, 512], F32, tag=tag, name=f"ps_y{b}_{r}")
            for kt in range(KD):
                nc.tensor.matmul(
                    ps_y,
                    osb[b][:, kt, 128 * r:128 * r + 128],
                    wout_sb[:, kt, :],
                    start=(kt == 0), stop=(kt == KD - 1),
                )
            y_sb = p_y.tile([128, D], F32, tag="y")
            nc.vector.scalar_tensor_tensor(
                out=y_sb, in0=ps_y, scalar=1.0, in1=bout_bc,
                op0=MUL, op1=ADD)
            bal.add("dve", 0.69)
            nc.sync.dma_start(out=y[b, 128 * r:128 * r + 128, :], in_=y_sb)

        # ---------------- attention rounds ----------------
        def emit_scores(b, g, jt):
            he, ho = 2 * g, 2 * g + 1
            s = {}
            if V3_PIH:
                # alloc order matches engine finish order (he->ACT, ho->DVE,
                # each engine does ih0 then ih1) so ring slots free earliest
                for h, ih in ((he, 0), (ho, 0), (he, 1), (ho, 1)):
                    s[(h, ih)] = psA.tile([128, 512], F32, tag="a",
                                          name=f"ps_s{b}_{h}_{jt}_{ih}")
            else:
                for h in (he, ho):
                    t = psA.tile([128, 1024], F32, tag="a",
                                 name=f"ps_s{b}_{h}_{jt}")
                    s[(h, 0)] = t[:, 0:512]
                    s[(h, 1)] = t[:, 512:1024]
            kt_he = qkT[b][0:64, 4 + g, 128 * jt:128 * jt + 128]
            kt_ho = qkT[b][64:128, 4 + g, 128 * jt:128 * jt + 128]
            # alternate row groups for PE-internal concurrency
            for ih in range(2):
                q_he = qkT[b][0:64, g, 512 * ih:512 * ih + 512]
                q_ho = qkT[b][64:128, g, 512 * ih:512 * ih + 512]
                nc.tensor.matmul(s[(he, ih)], kt_he, q_he,
                                 start=True, stop=True)
                nc.tensor.matmul(s[(ho, ih)], kt_ho, q_ho,
                                 start=True, stop=True)
            return s

        def emit_exp(b, g, jt, s):
            he, ho = 2 * g, 2 * g + 1
            expts = {}
            if V3_PIH:
                # per-head engine (precision: a whole softmax row keeps one
                # exp flavor so errors cancel in the denominator)
                for ih in range(2):
                    expT = p_exp.tile([128, 512], BF16, tag="exp",
                                      name=f"expT{b}_{he}_{jt}_{ih}")
                    nc.scalar.activation(expT, s[(he, ih)], EXP,
                                         scale=scale_sb[:, he:he + 1])
                    bal.add("act", 0.78)
                    expts[(he, ih)] = expT
                for ih in range(2):
                    expT = p_exp.tile([128, 512], BF16, tag="exp",
                                      name=f"expT{b}_{ho}_{jt}_{ih}")
                    nc.vector.tensor_scalar(
                        out=expT.bitcast(I16), in0=s[(ho, ih)],
                        scalar1=scale_schr[:, ho:ho + 1], scalar2=float(B16),
                        op0=MUL, op1=ADD,
                    )
                    bal.add("dve", 0.74)
                    expts[(ho, ih)] = expT
            else:
                for h in (he, ho):
                    expT = p_exp.tile([128, 1024], BF16, tag="exp", bufs=8,
                                      name=f"expT{b}_{h}_{jt}")
                    if h == he:
                        nc.scalar.activation(expT[:, 0:512], s[(h, 0)], EXP,
                                             scale=scale_sb[:, h:h + 1])
                        nc.scalar.activation(expT[:, 512:1024], s[(h, 1)], EXP,
                                             scale=scale_sb[:, h:h + 1])
                        bal.add("act", 1.14)
                    else:
                        nc.vector.tensor_scalar(
                            out=expT[:, 0:512].bitcast(I16), in0=s[(h, 0)],
                            scalar1=scale_schr[:, h:h + 1], scalar2=float(B16),
                            op0=MUL, op1=ADD)
                        nc.vector.tensor_scalar(
                            out=expT[:, 512:1024].bitcast(I16), in0=s[(h, 1)],
                            scalar1=scale_schr[:, h:h + 1], scalar2=float(B16),
                            op0=MUL, op1=ADD)
                        bal.add("dve", 1.3)
                    expts[(h, 0)] = expT[:, 0:512]
                    expts[(h, 1)] = expT[:, 512:1024]
            # diagonal self-token mask: zero the [128,128] diag block.
            # On GpSimd so the DVE queue stays exp-only (ring slot frees
            # gate next round's score pairs); attnV lag-3 absorbs the
            # slower Pool queue.
            ihd = jt // 4
            off = 128 * (jt % 4)
            for h in (he, ho):
                t = expts[(h, ihd)]
                nc.gpsimd.tensor_tensor(
                    t[:, off:off + 128], t[:, off:off + 128], inv_eye, op=MUL)
            return expts

        ps_os = {}

        def emit_attnv(b, g, jt, expts):
            for h in (2 * g, 2 * g + 1):
                if jt == 0:
                    rows = 128 if V3_PADV else DH + 1
                    if V3_PSO:
                        for ih in range(2):
                            ps_os[(h, ih)] = psO.tile(
                                [rows, 512], F32, tag="o",
                                name=f"ps_o{b}_{h}_{ih}")
                    else:
                        t = psO.tile([rows, N], F32, tag="o",
                                     name=f"ps_o{b}_{h}")
                        ps_os[(h, 0)] = t[:, 0:512]
                        ps_os[(h, 1)] = t[:, 512:1024]
                st = vsb[b]
                e0, e1 = expts[(h, 0)], expts[(h, 1)]
                o0, o1 = ps_os[(h, 0)], ps_os[(h, 1)]
                last = (jt == NT - 1)
                if V3_SPLIT:
                    if V3_PADV:
                        l0 = st[0:64, jt, h, :]      # j-half 0 -> rows 0-63
                        l1 = st[64:128, jt, h, :]    # j-half 1 -> rows 64-127
                    else:
                        l0 = st[0:64, jt, VW * h:VW * h + DH + 1]
                        l1 = st[64:128, jt, VW * h:VW * h + DH + 1]
                    # pair across banks: (j0->b0 || j1->b1), (j0->b1 || j1->b0)
                    nc.tensor.matmul(o0, l0, e0[0:64, :],
                                     start=(jt == 0), stop=False)
                    nc.tensor.matmul(o1, l1, e1[64:128, :],
                                     start=(jt == 0), stop=False)
                    nc.tensor.matmul(o1, l0, e1[0:64, :],
                                     start=False, stop=last)
                    nc.tensor.matmul(o0, l1, e0[64:128, :],
                                     start=False, stop=last)
                else:
                    if V3_PADV:
                        lf = st[:, jt, h, :]
                    else:
                        lf = st[:, jt, VW * h:VW * h + DH + 1]
                    nc.tensor.matmul(o0, lf, e0,
                                     start=(jt == 0), stop=last)
                    nc.tensor.matmul(o1, lf, e1,
                                     start=(jt == 0), stop=last)

        def fp_head(b, g, h, tail=False):
            """Evacuate one head's psO (both ih), bounce the denominator
            through DRAM for a partition-spread reciprocal, broadcast."""
            o_un = p_on.tile([DH + 1, N], F32, tag="on", name=f"o_un{b}_{h}")
            scrB = p_dram.tile([1, N], F32, tag="scrB", name=f"scrB{b}_{h}")
            if tail:
                # latency-optimized: per-ih pipelined single bounce so the
                # first token half reaches osb (and tail yproj) early
                r_row = p_sm.tile([1, N], F32, tag="rrow", name=f"rrow{b}_{h}")
                rb = p_rb.tile([64, N], F32, tag="rb", name=f"rb{b}_{h}")
                for ih in range(2):
                    lo = 512 * ih
                    if ih == 0:
                        nc.scalar.copy(o_un[:, lo:lo + 512],
                                       ps_os[(h, ih)][0:DH + 1, :])
                    else:
                        nc.vector.tensor_copy(o_un[:, lo:lo + 512],
                                              ps_os[(h, ih)][0:DH + 1, :])
                    nc.vector.reciprocal(r_row[:, lo:lo + 512],
                                         o_un[DH:DH + 1, lo:lo + 512])
                    nc.sync.dma_start(out=scrB[:, lo:lo + 512],
                                      in_=r_row[:, lo:lo + 512])
                    nc.sync.dma_start(
                        out=rb[:, lo:lo + 512],
                        in_=bass.AP(tensor=scrB.tensor,
                                    offset=scrB.offset + lo,
                                    ap=[[0, 64], [1, 512]]),
                    )
                return o_un, rb
            for ih in range(2):
                evac(o_un[:, 512 * ih:512 * ih + 512],
                     ps_os[(h, ih)][0:DH + 1, :], cost_scale=1.0)
            if True:
                scrA = p_dram.tile([1, N], F32, tag="scrA", name=f"scrA{b}_{h}")
                nc.sync.dma_start(out=scrA, in_=o_un[DH:DH + 1, :])
                s128 = p_sm.tile([128, N // 128], F32, tag="s128",
                                 name=f"s128_{b}_{h}")
                nc.sync.dma_start(
                    out=s128,
                    in_=bass.AP(tensor=scrA.tensor, offset=scrA.offset,
                                ap=[[N // 128, 128], [1, N // 128]]),
                )
                r128 = p_sm.tile([128, N // 128], F32, tag="r128",
                                 name=f"r128_{b}_{h}")
                nc.vector.reciprocal_approx_fast(r128, s128)
                bal.add("dve", 0.1)
                nc.sync.dma_start(
                    out=bass.AP(tensor=scrB.tensor, offset=scrB.offset,
                                ap=[[N // 128, 128], [1, N // 128]]),
                    in_=r128,
                )
            rb = p_rb.tile([64, N], F32, tag="rb", name=f"rb{b}_{h}")
            nc.sync.dma_start(
                out=rb,
                in_=bass.AP(tensor=scrB.tensor, offset=scrB.offset,
                            ap=[[0, 64], [1, N]]),
            )
            return o_un, rb

        def fp_norm(b, g, h, o_un, rb, tail=False):
            # NOTE: a partition-shifted osb write (osb[64:128] <-
            # o_un[0:64] * rb[0:64]) only works on GpSimd; DVE silently
            # corrupts.  Pair g=3 packs heads swapped (ho | he) to match
            # the swapped wout_sb kt=3 halves, so the tail pair's h=7
            # write is shift-free and can run on DVE in parallel with
            # h=6 on GpSimd.
            if g == KD - 1:
                q_off = ((h + 1) % 2) * 64
            else:
                q_off = (h % 2) * 64
            if tail:
                # per-ih halves so tail yproj r<4 starts after half 0
                eng = nc.vector if q_off == 0 else nc.gpsimd
                for ih in range(2):
                    lo = 512 * ih
                    eng.tensor_tensor(
                        osb[b][q_off:q_off + 64, g, lo:lo + 512],
                        o_un[0:DH, lo:lo + 512], rb[:, lo:lo + 512], op=MUL)
                if q_off == 0:
                    bal.add("dve", 1.3)
            else:
                nc.gpsimd.tensor_tensor(
                    osb[b][q_off:q_off + 64, g, :], o_un[0:DH, :], rb, op=MUL)

        # ================= emission schedule =================
        import functools
        import heapq
        F = functools.partial

        # ---- prologue ----
        x0 = emit_load_x(0)
        emit_const_dmas_early()
        alloc_batch(0)
        for half in range(2):
            for kd in range(KD):
                u_tr(0, x0, kd, half)
        emit_vsb_init(0)
        for nh in range(2):
            u_qk(0, 0, nh)
            u_qk(0, 4, nh)
        u_v(0, 0)
        u_v(0, 1)

        # ---- filler queue: (deadline_round, seq, emit_fn) ----
        q = []
        _seq = [0]

        def push(dl, fn):
            heapq.heappush(q, (dl, _seq[0], fn))
            _seq[0] += 1

        for r in range(2, NT):                        # v(0) r2..7
            push(r - 1, F(u_v, 0, r))
        for p, (ctq, ctk) in enumerate(((1, 5), (2, 6), (3, 7)), start=1):
            push(8 * p - 6, F(u_qk, 0, ctq, 0))
            push(8 * p - 5, F(u_qk, 0, ctq, 1))
            push(8 * p - 4, F(u_qk, 0, ctk, 0))
            push(8 * p - 3, F(u_qk, 0, ctk, 1))

        x1 = [None]

        def start_b1_load():
            x1[0] = emit_load_x(1)
            emit_const_dmas_late()
            alloc_batch(1)

        push(1, start_b1_load)
        for i, (kd, half) in enumerate(
                [(kd, half) for half in range(2) for kd in range(KD)]):
            push(6 + i, F(lambda kd=kd, half=half: u_tr(1, x1[0], kd, half)))
        push(14, F(emit_vsb_init, 1))
        push(22, F(u_v, 1, 0))
        push(23, F(u_v, 1, 1))
        push(25, F(u_qk, 1, 0, 0))
        push(26, F(u_qk, 1, 0, 1))
        push(27, F(u_qk, 1, 4, 0))
        push(28, F(u_qk, 1, 4, 1))
        for i, r in enumerate(range(2, NT)):          # v(1) r2..7
            push(29 + i // 2, F(u_v, 1, r))
        for p, (ctq, ctk) in enumerate(((1, 5), (2, 6), (3, 7)), start=1):
            push(32 + 8 * p - 6, F(u_qk, 1, ctq, 0))
            push(32 + 8 * p - 5, F(u_qk, 1, ctq, 1))
            push(32 + 8 * p - 4, F(u_qk, 1, ctk, 0))
            push(32 + 8 * p - 3, F(u_qk, 1, ctk, 1))
        for i in range(NT):                            # yproj(0)
            push(39 + 2 * i, F(u_yp, 0, i))

        def emit_finish_pair(b, g, tail=False, defer=None):
            he, ho = 2 * g, 2 * g + 1
            if tail or defer is None:
                fins = []
                for h in (he, ho):
                    fins.append((h,) + fp_head(b, g, h, tail=False))
                for h, o_un, rb in fins:
                    fp_norm(b, g, h, o_un, rb, tail=tail)
                return
            # spread the pair-boundary evac/recip spike over two rounds:
            # head-even now, head-odd next round (psO ring has 3 rounds of
            # slack at attnV lag-3)
            o_un, rb = fp_head(b, g, he)
            fp_norm(b, g, he, o_un, rb)

            def rest():
                o2, rb2 = fp_head(b, g, ho)
                fp_norm(b, g, ho, o2, rb2)
            defer(rest)

        # ---- attention rounds with lag-3 attnV ----
        rounds = [(b, g, jt) for b in range(BPC) for g in range(4)
                  for jt in range(NT)]
        import collections
        pend = collections.deque()   # lag-3 attnV: consume exps 3 rounds back
        for ridx, (b, g, jt) in enumerate(rounds):
            s_tiles = emit_scores(b, g, jt)
            expts = emit_exp(b, g, jt, s_tiles)
            while q and q[0][0] <= ridx:
                heapq.heappop(q)[2]()
            if len(pend) == 3:
                pb, pg, pjt, pexp = pend.popleft()
                emit_attnv(pb, pg, pjt, pexp)
                if pjt == NT - 1:
                    emit_finish_pair(
                        pb, pg,
                        defer=lambda fn, r=ridx: push(r + 1, fn))
            pend.append((b, g, jt, expts))
        while pend:
            pb, pg, pjt, pexp = pend.popleft()
            emit_attnv(pb, pg, pjt, pexp)
            if pjt == NT - 1 and pend:
                emit_finish_pair(pb, pg)
        while q:
            heapq.heappop(q)[2]()
        emit_finish_pair(pb, pg, tail=True)

        # ---- tail: yproj(1), alternating psum rings for depth ----
        for r in range(NT):
            u_yp(1, r, pool=(psA if r % 2 == 0 else psO))

    nc.compile()
    return nc


_NC = None


def _get_program():
    global _NC
    if _NC is None:
        _NC = build_program()
    return _NC


def make_in_maps(x, w_qkv, w_out, b_out, scale):
    x = np.ascontiguousarray(np.asarray(x, dtype=np.float32))
    w_qkv = np.ascontiguousarray(np.asarray(w_qkv, dtype=np.float32))
    w_out = np.ascontiguousarray(np.asarray(w_out, dtype=np.float32))
    b_out = np.ascontiguousarray(np.asarray(b_out, dtype=np.float32))
    scale = np.ascontiguousarray(np.asarray(scale, dtype=np.float32))
    return [
        {
            "x": x[c * BPC:(c + 1) * BPC],
            "w_qkv": w_qkv,
            "w_out": w_out,
            "b_out": b_out,
            "scale": scale,
        }
        for c in range(N_CORES)
    ]


def kernel(x, w_qkv, w_out, b_out, scale):
    nc = _get_program()
    in_maps = make_in_maps(x, w_qkv, w_out, b_out, scale)
    res = run_bass_kernel_spmd(nc, in_maps, core_ids=list(range(N_CORES)))
    return np.concatenate([res.results[c]["y"] for c in range(N_CORES)], axis=0)


if __name__ == "__main__":
    rng = np.random.default_rng(0)
    inputs = {
        "x": rng.standard_normal((B, N, D), dtype=np.float32),
        "w_qkv": rng.standard_normal((D, 3 * D), dtype=np.float32) * 0.03,
        "w_out": rng.standard_normal((D, D), dtype=np.float32) * 0.04,
        "b_out": np.zeros(D, dtype=np.float32),
        "scale": np.full(HEADS, DH ** -0.5, dtype=np.float32),
    }
    out = kernel(**inputs)
    print("kernel output", out.shape, out.dtype)


# revision 27
# speedup vs baseline: 1.0046x; 1.0046x over previous
"""Trainium2 Bass kernel for nn_Attention_1503238553757 (LSA attention) — v3.

Reference computation (per batch element):
    qkv = x @ w_qkv; q,k,v heads of dim 64
    dots = (q @ k^T) * scale[h]; diagonal masked to -inf
    attn = softmax(dots); out = attn @ v
    y = concat_heads(out) @ w_out + b_out

Sharding: data-parallel over batch (16 batches -> 2 per core x 8 cores).

v3 changes vs v2 (258us baseline):
  - PSUM at 1-bank granularity: scores are four [128,512] f32 tiles per
    round (per head x ih) in a shared 4-slot ring (psA) that filler
    evac tiles also ride; attnV accumulators are four [128,512] tiles
    (psO ring).  The v2 ring-of-2 shared by scores+fillers serialized
    the whole pipeline (measured 3.07us rounds, MM issue gaps ~290ns).
  - True PE row-group concurrency: score MMs alternate row groups
    (head-even at rows 0-63, head-odd at 64-127) with adjacent queue
    positions so pairs run concurrently; attnV is split into K=64
    j-halves that pair across different psum banks (never same-bank
    concurrent) for the same 2x effect.
  - attnV stationary padded to 128 cols (v | ones | zeros) -> FWL
    weight loads; psum rows 65-127 are garbage and never read.
  - exp ops are per-ih [128,512]: ACT true-exp for head-even, DVE
    Schraudolph for head-odd; psum slots free in ~600ns.
  - yproj evac uses scalar_tensor_tensor (psum + bias in one DVE op).
"""

import os
import sys

for _p in ("/opt/trn_rl_repo", "/root/.axon_site/_ro/trn_rl_repo"):
    if os.path.isdir(_p) and _p not in sys.path:
        sys.path.insert(0, _p)

import numpy as np

import concourse.bass as bass
import concourse.bacc as bacc
import concourse.tile as tile
import concourse.mybir as mybir
from concourse.bass_utils import run_bass_kernel_spmd

# Problem constants (hardcoded per harness contract)
B, N, D = 16, 1024, 512
HEADS, DH = 8, 64
N_CORES = 8
BPC = B // N_CORES  # batches per core = 2

dt = mybir.dt
F32 = dt.float32
BF16 = dt.bfloat16
F16 = dt.float16
I16 = dt.int16
EXP = mybir.ActivationFunctionType.Exp
MUL = mybir.AluOpType.mult
ADD = mybir.AluOpType.add

NT = N // 128   # token tiles = 8
KD = D // 128   # d/inner k-tiles = 4

# Schraudolph bf16-exp constants (DVE f32->int16 is round-to-nearest):
# exp(x) ~= bitcast_bf16(int16(A16*x + B16))
A16 = 128.0 / float(np.log(2.0))     # 184.6650
B16 = 127.0 * 128.0 - 7.4115         # 16248.59

# debug knobs for bisecting
V3_STT = os.environ.get("V3_STT", "0") == "1"
V3_SPLIT = os.environ.get("V3_SPLIT", "0") == "1"
V3_PIH = os.environ.get("V3_PIH", "1") == "1"  # per-ih scores/exp tiles
V3_PADV = os.environ.get("V3_PADV", "1") == "1"  # padded [*,128] v stationary
V3_PSO = os.environ.get("V3_PSO", "1") == "1"    # per-ih psO accumulators
VW = DH + 1


class EngBal:
    """Static load balancer between the Scalar (act) and Vector (dve)
    engines for PSUM-consuming ops."""

    def __init__(self, nc):
        self.nc = nc
        self.t = {"act": 0.0, "dve": 0.0}

    def pick(self, cost_act, cost_dve):
        if self.t["act"] + cost_act <= self.t["dve"] + cost_dve:
            self.t["act"] += cost_act
            return "act"
        self.t["dve"] += cost_dve
        return "dve"

    def add(self, eng, cost):
        self.t[eng] += cost


def build_program():
    nc = bacc.Bacc("TRN2", target_bir_lowering=False, debug=False,
                   num_devices=N_CORES)

    x = nc.dram_tensor("x", [BPC, N, D], F32, kind="ExternalInput").ap()
    w_qkv = nc.dram_tensor("w_qkv", [D, 3 * D], F32, kind="ExternalInput").ap()
    w_out = nc.dram_tensor("w_out", [D, D], F32, kind="ExternalInput").ap()
    b_out = nc.dram_tensor("b_out", [D], F32, kind="ExternalInput").ap()
    scale = nc.dram_tensor("scale", [HEADS], F32, kind="ExternalInput").ap()
    y = nc.dram_tensor("y", [BPC, N, D], F32, kind="ExternalOutput").ap()

    ident_dram = nc.inline_tensor(np.eye(128, dtype=np.float16), name="ident")
    ident32_dram = nc.inline_tensor(np.eye(128, dtype=np.float32),
                                    name="ident32")

    bal = EngBal(nc)

    import contextlib
    with tile.TileContext(nc) as tc, contextlib.ExitStack() as ctx:
        consts = ctx.enter_context(tc.tile_pool(name="consts", bufs=1))
        p_x = ctx.enter_context(tc.tile_pool(name="p_x", bufs=1))
        p_big = ctx.enter_context(tc.tile_pool(name="p_big", bufs=2))
        p_exp = ctx.enter_context(tc.tile_pool(name="p_exp", bufs=22))
        p_on = ctx.enter_context(tc.tile_pool(name="p_on", bufs=4))
        p_rb = ctx.enter_context(tc.tile_pool(name="p_rb", bufs=4))
        p_sm = ctx.enter_context(tc.tile_pool(name="p_sm", bufs=4))
        p_y = ctx.enter_context(tc.tile_pool(name="p_y", bufs=3))
        psA = ctx.enter_context(tc.tile_pool(name="psA", bufs=(4 if V3_PIH else 2),
                                             space="PSUM"))
        psO = ctx.enter_context(tc.tile_pool(name="psO", bufs=(4 if V3_PSO else 2),
                                             space="PSUM"))
        p_dram = ctx.enter_context(tc.tile_pool(name="p_dram", bufs=4,
                                                space="DRAM"))

        # ---------------- constants ----------------
        ident_sb = consts.tile([128, 128], F16)
        nc.sync.dma_start(out=ident_sb, in_=ident_dram.ap())
        inv_eye = consts.tile([128, 128], BF16)
        nc.vector.tensor_scalar(out=inv_eye, in0=ident_sb, scalar1=-1.0,
                                scalar2=1.0, op0=MUL, op1=ADD)
        wqkv_sb = consts.tile([128, KD, 3 * D], F16)
        wout_sb = consts.tile([128, KD, D], F16)
        bout_bc = consts.tile([128, D], F32)
        scale_sb = consts.tile([128, HEADS], F32)
        scale_schr = consts.tile([128, HEADS], F32)

        def emit_const_dmas_early():
            # q/k columns of w_qkv first (prologue critical path)
            nc.gpsimd.dma_start(
                out=wqkv_sb[:, :, 0:2 * D],
                in_=w_qkv.rearrange("(k p) c -> p k c", p=128)[:, :, 0:2 * D],
            )
            nc.gpsimd.dma_start(
                out=wqkv_sb[:, :, 2 * D:3 * D],
                in_=w_qkv.rearrange("(k p) c -> p k c", p=128)[:, :, 2 * D:3 * D],
            )
            nc.sync.dma_start(
                out=bout_bc,
                in_=bass.AP(tensor=b_out.tensor, offset=0,
                            ap=[[0, 128], [1, D]]),
            )
            nc.sync.dma_start(
                out=scale_sb,
                in_=bass.AP(tensor=scale.tensor, offset=0,
                            ap=[[0, 128], [1, HEADS]]),
            )
            nc.vector.tensor_scalar_mul(scale_schr, scale_sb, float(A16))

        def emit_const_dmas_late():
            # kt=3 halves swapped so pair g=3 packs (ho | he) in osb;
            # lets the tail normalize run shift-free on DVE for head 7.
            src = w_out.rearrange("(k p) c -> p k c", p=128)
            nc.gpsimd.dma_start(out=wout_sb[:, 0:KD - 1, :],
                                in_=src[:, 0:KD - 1, :])
            nc.gpsimd.dma_start(out=wout_sb[0:64, KD - 1, :],
                                in_=src[64:128, KD - 1, :])
            nc.gpsimd.dma_start(out=wout_sb[64:128, KD - 1, :],
                                in_=src[0:64, KD - 1, :])

        # ---------------- per-batch state ----------------
        xT = [None] * BPC
        qkT = [None] * BPC
        vsb = [None] * BPC   # [128, NT, HEADS, 128]: v | ones | zeros
        osb = [None] * BPC

        def alloc_batch(b):
            xT[b] = p_big.tile([128, KD, N], F16, tag="xT", name=f"xT{b}")
            qkT[b] = p_big.tile([128, 8, N], F16, tag="qk", name=f"qkT{b}")
            if V3_PADV:
                vsb[b] = p_big.tile([128, NT, HEADS, 128], BF16, tag="v",
                                    name=f"v{b}")
            else:
                vsb[b] = p_big.tile([128, NT, HEADS * VW + 64], BF16, tag="v",
                                    name=f"v{b}")
            osb[b] = p_big.tile([128, KD, N], F16, tag="o", name=f"o{b}")

        def emit_load_x(b, chunks=range(4), x_sb=None):
            # f32->f16 casting DMAs must go through the gpsimd queue; the
            # serial queue also keeps chunk 0 (prologue critical path)
            # ahead of the weight streams in DMA bandwidth.
            if x_sb is None:
                x_sb = p_x.tile([128, NT, D], F16, tag="x", name=f"x_sb{b}")
            src = x[b].rearrange("(r p) d -> p r d", p=128)
            for c in chunks:
                nc.gpsimd.dma_start(out=x_sb[:, 2 * c:2 * c + 2, :],
                                    in_=src[:, 2 * c:2 * c + 2, :])
            return x_sb

        def emit_vsb_init(b):
            if V3_PADV:
                # ones column at DH, zeros at DH+1.. (padded stationary cols)
                nc.gpsimd.memset(vsb[b][:, :, :, DH:DH + 1], 1.0)
                nc.gpsimd.memset(vsb[b][:, :, :, DH + 1:], 0.0)
            else:
                nc.gpsimd.memset(
                    vsb[b][:, :, 0:HEADS * VW].rearrange(
                        "p r (h e) -> p r h e", h=HEADS)[:, :, :, DH:DH + 1],
                    1.0,
                )
                nc.gpsimd.memset(vsb[b][:, :, HEADS * VW:], 1.0)

        # ---------------- filler units (ride the psA ring) ----------------
        def evac(dst_ap, src_ap, cost_scale=1.0, eng=None):
            # In-round filler evacs are pinned to ACT: its two exps finish
            # ~2.3us into the round, so the evac lands ~2.9us -- just in
            # time for the next round's 4th score MM, which recycles the
            # filler's psA slot (the 5-allocs-on-4-slots wraparound victim).
            e = eng or bal.pick(0.57 * cost_scale, 0.66 * cost_scale)
            if e == "act":
                bal.add("act", 0.57 * cost_scale) if eng else None
                nc.scalar.copy(dst_ap, src_ap)
            else:
                nc.vector.tensor_copy(dst_ap, src_ap)

        def u_tr(b, x_sb, kd, half):
            ps_t = psA.tile([128, 512], F16, tag="a",
                            name=f"ps_t{b}_{kd}_{half}")
            for rr in range(4):
                r = 4 * half + rr
                nc.tensor.transpose(
                    ps_t[:, 128 * rr:128 * rr + 128],
                    x_sb[:, r, 128 * kd:128 * kd + 128],
                    ident_sb,
                )
            evac(xT[b][:, kd, 512 * half:512 * half + 512], ps_t)

        def u_qk(b, ct, nh):
            ps_qk = psA.tile([128, 512], F32, tag="a",
                             name=f"ps_qk{b}_{ct}_{nh}")
            for kt in range(KD):
                nc.tensor.matmul(
                    ps_qk,
                    wqkv_sb[:, kt, 128 * ct:128 * ct + 128],
                    xT[b][:, kt, 512 * nh:512 * nh + 512],
                    start=(kt == 0), stop=(kt == KD - 1),
                )
            evac(qkT[b][:, ct, 512 * nh:512 * nh + 512], ps_qk)

        def u_v(b, r):
            ps_v = psA.tile([128, 512], F32, tag="a", name=f"ps_v{b}_{r}")
            for kt in range(KD):
                nc.tensor.matmul(
                    ps_v,
                    xT[b][:, kt, 128 * r:128 * r + 128],
                    wqkv_sb[:, kt, 2 * D:3 * D],
                    start=(kt == 0), stop=(kt == KD - 1),
                )
            if V3_PADV:
                evac(
                    vsb[b][:, r, :, 0:DH],
                    ps_v.rearrange("p (h e) -> p h e", h=HEADS),
                )
            else:
                evac(
                    vsb[b][:, r, 0:HEADS * VW].rearrange(
                        "p (h e) -> p h e", h=HEADS)[:, :, 0:DH],
                    ps_v.rearrange("p (h e) -> p h e", h=HEADS),
                )

        def u_yp(b, r, pool=None):
            pool = pool if pool is not None else psA
            tag = "a" if pool is psA else "o"
            ps_y = pool.tile([128, 512], F32, tag=tag, name=f"ps_y{b}_{r}")
            for kt in range(KD):
                nc.tensor.matmul(
                    ps_y,
                    osb[b][:, kt, 128 * r:128 * r + 128],
                    wout_sb[:, kt, :],
                    start=(kt == 0), stop=(kt == KD - 1),
                )
            y_sb = p_y.tile([128, D], F32, tag="y")
            nc.vector.scalar_tensor_tensor(
                out=y_sb, in0=ps_y, scalar=1.0, in1=bout_bc,
                op0=MUL, op1=ADD)
            bal.add("dve", 0.69)
            # batch-1 (tail) y DMAs alternate sync/scalar queues: ACT is
            # idle at the tail and this halves the issue serialization.
            # batch-0's stay on sync (ACT is critical during rounds).
            eng = nc.scalar if (b == 1 and r % 2 == 1) else nc.sync
            eng.dma_start(out=y[b, 128 * r:128 * r + 128, :], in_=y_sb)

        # ---------------- attention rounds ----------------
        def emit_scores(b, g, jt):
            he, ho = 2 * g, 2 * g + 1
            s = {}
            if V3_PIH:
                # alloc order matches engine finish order (he->ACT, ho->DVE,
                # each engine does ih0 then ih1) so ring slots free earliest
                for h, ih in ((he, 0), (ho, 0), (he, 1), (ho, 1)):
                    s[(h, ih)] = psA.tile([128, 512], F32, tag="a",
                                          name=f"ps_s{b}_{h}_{jt}_{ih}")
            else:
                for h in (he, ho):
                    t = psA.tile([128, 1024], F32, tag="a",
                                 name=f"ps_s{b}_{h}_{jt}")
                    s[(h, 0)] = t[:, 0:512]
                    s[(h, 1)] = t[:, 512:1024]
            kt_he = qkT[b][0:64, 4 + g, 128 * jt:128 * jt + 128]
            kt_ho = qkT[b][64:128, 4 + g, 128 * jt:128 * jt + 128]
            # alternate row groups for PE-internal concurrency
            for ih in range(2):
                q_he = qkT[b][0:64, g, 512 * ih:512 * ih + 512]
                q_ho = qkT[b][64:128, g, 512 * ih:512 * ih + 512]
                nc.tensor.matmul(s[(he, ih)], kt_he, q_he,
                                 start=True, stop=True)
                nc.tensor.matmul(s[(ho, ih)], kt_ho, q_ho,
                                 start=True, stop=True)
            return s

        def emit_exp(b, g, jt, s):
            he, ho = 2 * g, 2 * g + 1
            expts = {}
            if V3_PIH:
                # per-head engine (precision: a whole softmax row keeps one
                # exp flavor so errors cancel in the denominator)
                for ih in range(2):
                    expT = p_exp.tile([128, 512], BF16, tag="exp",
                                      name=f"expT{b}_{he}_{jt}_{ih}")
                    nc.scalar.activation(expT, s[(he, ih)], EXP,
                                         scale=scale_sb[:, he:he + 1])
                    bal.add("act", 0.78)
                    expts[(he, ih)] = expT
                for ih in range(2):
                    expT = p_exp.tile([128, 512], BF16, tag="exp",
                                      name=f"expT{b}_{ho}_{jt}_{ih}")
                    nc.vector.tensor_scalar(
                        out=expT.bitcast(I16), in0=s[(ho, ih)],
                        scalar1=scale_schr[:, ho:ho + 1], scalar2=float(B16),
                        op0=MUL, op1=ADD,
                    )
                    bal.add("dve", 0.74)
                    expts[(ho, ih)] = expT
            else:
                for h in (he, ho):
                    expT = p_exp.tile([128, 1024], BF16, tag="exp", bufs=8,
                                      name=f"expT{b}_{h}_{jt}")
                    if h == he:
                        nc.scalar.activation(expT[:, 0:512], s[(h, 0)], EXP,
                                             scale=scale_sb[:, h:h + 1])
                        nc.scalar.activation(expT[:, 512:1024], s[(h, 1)], EXP,
                                             scale=scale_sb[:, h:h + 1])
                        bal.add("act", 1.14)
                    else:
                        nc.vector.tensor_scalar(
                            out=expT[:, 0:512].bitcast(I16), in0=s[(h, 0)],
                            scalar1=scale_schr[:, h:h + 1], scalar2=float(B16),
                            op0=MUL, op1=ADD)
                        nc.vector.tensor_scalar(
                            out=expT[:, 512:1024].bitcast(I16), in0=s[(h, 1)],
                            scalar1=scale_schr[:, h:h + 1], scalar2=float(B16),
                            op0=MUL, op1=ADD)
                        bal.add("dve", 1.3)
                    expts[(h, 0)] = expT[:, 0:512]
                    expts[(h, 1)] = expT[:, 512:1024]
            # diagonal self-token mask: zero the [128,128] diag block.
            # On GpSimd so the DVE queue stays exp-only (ring slot frees
            # gate next round's score pairs); attnV lag-3 absorbs the
            # slower Pool queue.
            ihd = jt // 4
            off = 128 * (jt % 4)
            for h in (he, ho):
                t = expts[(h, ihd)]
                nc.gpsimd.tensor_tensor(
                    t[:, off:off + 128], t[:, off:off + 128], inv_eye, op=MUL)
            return expts

        ps_os = {}

        def emit_attnv(b, g, jt, expts):
            for h in (2 * g, 2 * g + 1):
                if jt == 0:
                    rows = 128 if V3_PADV else DH + 1
                    if V3_PSO:
                        for ih in range(2):
                            ps_os[(h, ih)] = psO.tile(
                                [rows, 512], F32, tag="o",
                                name=f"ps_o{b}_{h}_{ih}")
                    else:
                        t = psO.tile([rows, N], F32, tag="o",
                                     name=f"ps_o{b}_{h}")
                        ps_os[(h, 0)] = t[:, 0:512]
                        ps_os[(h, 1)] = t[:, 512:1024]
                st = vsb[b]
                e0, e1 = expts[(h, 0)], expts[(h, 1)]
                o0, o1 = ps_os[(h, 0)], ps_os[(h, 1)]
                last = (jt == NT - 1)
                if V3_SPLIT:
                    if V3_PADV:
                        l0 = st[0:64, jt, h, :]      # j-half 0 -> rows 0-63
                        l1 = st[64:128, jt, h, :]    # j-half 1 -> rows 64-127
                    else:
                        l0 = st[0:64, jt, VW * h:VW * h + DH + 1]
                        l1 = st[64:128, jt, VW * h:VW * h + DH + 1]
                    # pair across banks: (j0->b0 || j1->b1), (j0->b1 || j1->b0)
                    nc.tensor.matmul(o0, l0, e0[0:64, :],
                                     start=(jt == 0), stop=False)
                    nc.tensor.matmul(o1, l1, e1[64:128, :],
                                     start=(jt == 0), stop=False)
                    nc.tensor.matmul(o1, l0, e1[0:64, :],
                                     start=False, stop=last)
                    nc.tensor.matmul(o0, l1, e0[64:128, :],
                                     start=False, stop=last)
                else:
                    if V3_PADV:
                        lf = st[:, jt, h, :]
                    else:
                        lf = st[:, jt, VW * h:VW * h + DH + 1]
                    nc.tensor.matmul(o0, lf, e0,
                                     start=(jt == 0), stop=last)
                    nc.tensor.matmul(o1, lf, e1,
                                     start=(jt == 0), stop=last)

        def fp_head(b, g, h, tail=False):
            """Evacuate one head's psO (both ih), bounce the denominator
            through DRAM for a partition-spread reciprocal, broadcast."""
            o_un = p_on.tile([DH + 1, N], F32, tag="on", name=f"o_un{b}_{h}")
            scrB = p_dram.tile([1, N], F32, tag="scrB", name=f"scrB{b}_{h}")
            if tail:
                # latency-optimized: per-ih pipelined single bounce so the
                # first token half reaches osb (and tail yproj) early
                r_row = p_sm.tile([1, N], F32, tag="rrow", name=f"rrow{b}_{h}")
                rb = p_rb.tile([64, N], F32, tag="rb", name=f"rb{b}_{h}")
                for ih in range(2):
                    lo = 512 * ih
                    if ih == 0:
                        nc.scalar.copy(o_un[:, lo:lo + 512],
                                       ps_os[(h, ih)][0:DH + 1, :])
                    else:
                        nc.vector.tensor_copy(o_un[:, lo:lo + 512],
                                              ps_os[(h, ih)][0:DH + 1, :])
                    nc.vector.reciprocal(r_row[:, lo:lo + 512],
                                         o_un[DH:DH + 1, lo:lo + 512])
                    nc.sync.dma_start(out=scrB[:, lo:lo + 512],
                                      in_=r_row[:, lo:lo + 512])
                    nc.sync.dma_start(
                        out=rb[:, lo:lo + 512],
                        in_=bass.AP(tensor=scrB.tensor,
                                    offset=scrB.offset + lo,
                                    ap=[[0, 64], [1, 512]]),
                    )
                return o_un, rb
            for ih in range(2):
                evac(o_un[:, 512 * ih:512 * ih + 512],
                     ps_os[(h, ih)][0:DH + 1, :], cost_scale=1.0)
            if True:
                scrA = p_dram.tile([1, N], F32, tag="scrA", name=f"scrA{b}_{h}")
                nc.sync.dma_start(out=scrA, in_=o_un[DH:DH + 1, :])
                s128 = p_sm.tile([128, N // 128], F32, tag="s128",
                                 name=f"s128_{b}_{h}")
                nc.sync.dma_start(
                    out=s128,
                    in_=bass.AP(tensor=scrA.tensor, offset=scrA.offset,
                                ap=[[N // 128, 128], [1, N // 128]]),
                )
                r128 = p_sm.tile([128, N // 128], F32, tag="r128",
                                 name=f"r128_{b}_{h}")
                nc.vector.reciprocal_approx_fast(r128, s128)
                bal.add("dve", 0.1)
                nc.sync.dma_start(
                    out=bass.AP(tensor=scrB.tensor, offset=scrB.offset,
                                ap=[[N // 128, 128], [1, N // 128]]),
                    in_=r128,
                )
            rb = p_rb.tile([64, N], F32, tag="rb", name=f"rb{b}_{h}")
            nc.sync.dma_start(
                out=rb,
                in_=bass.AP(tensor=scrB.tensor, offset=scrB.offset,
                            ap=[[0, 64], [1, N]]),
            )
            return o_un, rb

        def fp_norm(b, g, h, o_un, rb, tail=False):
            # NOTE: a partition-shifted osb write (osb[64:128] <-
            # o_un[0:64] * rb[0:64]) only works on GpSimd; DVE silently
            # corrupts.  Pair g=3 packs heads swapped (ho | he) to match
            # the swapped wout_sb kt=3 halves, so the tail pair's h=7
            # write is shift-free and can run on DVE in parallel with
            # h=6 on GpSimd.
            if g == KD - 1:
                q_off = ((h + 1) % 2) * 64
            else:
                q_off = (h % 2) * 64
            if tail:
                # per-ih halves so tail yproj r<4 starts after half 0
                eng = nc.vector if q_off == 0 else nc.gpsimd
                for ih in range(2):
                    lo = 512 * ih
                    eng.tensor_tensor(
                        osb[b][q_off:q_off + 64, g, lo:lo + 512],
                        o_un[0:DH, lo:lo + 512], rb[:, lo:lo + 512], op=MUL)
                if q_off == 0:
                    bal.add("dve", 1.3)
            else:
                nc.gpsimd.tensor_tensor(
                    osb[b][q_off:q_off + 64, g, :], o_un[0:DH, :], rb, op=MUL)

        # ================= emission schedule =================
        import functools
        import heapq
        F = functools.partial

        # ---- prologue ----
        x0 = emit_load_x(0)
        emit_const_dmas_early()
        alloc_batch(0)
        for half in range(2):
            for kd in range(KD):
                u_tr(0, x0, kd, half)
        emit_vsb_init(0)
        for nh in range(2):
            u_qk(0, 0, nh)
            u_qk(0, 4, nh)
        u_v(0, 0)
        u_v(0, 1)

        # ---- filler queue: (deadline_round, seq, emit_fn) ----
        q = []
        _seq = [0]

        def push(dl, fn):
            heapq.heappush(q, (dl, _seq[0], fn))
            _seq[0] += 1

        for r in range(2, NT):                        # v(0) r2..7
            push(r - 1, F(u_v, 0, r))
        for p, (ctq, ctk) in enumerate(((1, 5), (2, 6), (3, 7)), start=1):
            push(8 * p - 6, F(u_qk, 0, ctq, 0))
            push(8 * p - 5, F(u_qk, 0, ctq, 1))
            push(8 * p - 4, F(u_qk, 0, ctk, 0))
            push(8 * p - 3, F(u_qk, 0, ctk, 1))

        x1 = [None]

        def start_b1_load():
            x1[0] = emit_load_x(1)
            emit_const_dmas_late()
            alloc_batch(1)

        push(1, start_b1_load)
        for i, (kd, half) in enumerate(
                [(kd, half) for half in range(2) for kd in range(KD)]):
            push(6 + i, F(lambda kd=kd, half=half: u_tr(1, x1[0], kd, half)))
        push(14, F(emit_vsb_init, 1))
        push(22, F(u_v, 1, 0))
        push(23, F(u_v, 1, 1))
        push(25, F(u_qk, 1, 0, 0))
        push(26, F(u_qk, 1, 0, 1))
        push(27, F(u_qk, 1, 4, 0))
        push(28, F(u_qk, 1, 4, 1))
        for i, r in enumerate(range(2, NT)):          # v(1) r2..7
            push(29 + i // 2, F(u_v, 1, r))
        for p, (ctq, ctk) in enumerate(((1, 5), (2, 6), (3, 7)), start=1):
            push(32 + 8 * p - 6, F(u_qk, 1, ctq, 0))
            push(32 + 8 * p - 5, F(u_qk, 1, ctq, 1))
            push(32 + 8 * p - 4, F(u_qk, 1, ctk, 0))
            push(32 + 8 * p - 3, F(u_qk, 1, ctk, 1))
        for i in range(NT):                            # yproj(0)
            push(39 + 2 * i, F(u_yp, 0, i))

        def emit_finish_pair(b, g, tail=False, defer=None):
            he, ho = 2 * g, 2 * g + 1
            if tail or defer is None:
                fins = []
                for h in (he, ho):
                    fins.append((h,) + fp_head(b, g, h, tail=False))
                for h, o_un, rb in fins:
                    fp_norm(b, g, h, o_un, rb, tail=tail)
                return
            # spread the pair-boundary evac/recip spike over two rounds:
            # head-even now, head-odd next round (psO ring has 3 rounds of
            # slack at attnV lag-3)
            o_un, rb = fp_head(b, g, he)
            fp_norm(b, g, he, o_un, rb)

            def rest():
                o2, rb2 = fp_head(b, g, ho)
                fp_norm(b, g, ho, o2, rb2)
            defer(rest)

        # ---- attention rounds with lag-3 attnV ----
        rounds = [(b, g, jt) for b in range(BPC) for g in range(4)
                  for jt in range(NT)]
        import collections
        pend = collections.deque()   # lag-3 attnV: consume exps 3 rounds back
        for ridx, (b, g, jt) in enumerate(rounds):
            s_tiles = emit_scores(b, g, jt)
            expts = emit_exp(b, g, jt, s_tiles)
            while q and q[0][0] <= ridx:
                heapq.heappop(q)[2]()
            if len(pend) == 3:
                pb, pg, pjt, pexp = pend.popleft()
                emit_attnv(pb, pg, pjt, pexp)
                if pjt == NT - 1:
                    emit_finish_pair(
                        pb, pg,
                        defer=lambda fn, r=ridx: push(r + 1, fn))
            pend.append((b, g, jt, expts))
        while pend:
            pb, pg, pjt, pexp = pend.popleft()
            emit_attnv(pb, pg, pjt, pexp)
            if pjt == NT - 1 and pend:
                emit_finish_pair(pb, pg)
        while q:
            heapq.heappop(q)[2]()
        emit_finish_pair(pb, pg, tail=True)

        # ---- tail: yproj(1), alternating psum rings for depth ----
        for r in range(NT):
            u_yp(1, r, pool=(psA if r % 2 == 0 else psO))

    nc.compile()
    return nc


_NC = None


def _get_program():
    global _NC
    if _NC is None:
        _NC = build_program()
    return _NC


def make_in_maps(x, w_qkv, w_out, b_out, scale):
    x = np.ascontiguousarray(np.asarray(x, dtype=np.float32))
    w_qkv = np.ascontiguousarray(np.asarray(w_qkv, dtype=np.float32))
    w_out = np.ascontiguousarray(np.asarray(w_out, dtype=np.float32))
    b_out = np.ascontiguousarray(np.asarray(b_out, dtype=np.float32))
    scale = np.ascontiguousarray(np.asarray(scale, dtype=np.float32))
    return [
        {
            "x": x[c * BPC:(c + 1) * BPC],
            "w_qkv": w_qkv,
            "w_out": w_out,
            "b_out": b_out,
            "scale": scale,
        }
        for c in range(N_CORES)
    ]


def kernel(x, w_qkv, w_out, b_out, scale):
    nc = _get_program()
    in_maps = make_in_maps(x, w_qkv, w_out, b_out, scale)
    res = run_bass_kernel_spmd(nc, in_maps, core_ids=list(range(N_CORES)))
    return np.concatenate([res.results[c]["y"] for c in range(N_CORES)], axis=0)


if __name__ == "__main__":
    rng = np.random.default_rng(0)
    inputs = {
        "x": rng.standard_normal((B, N, D), dtype=np.float32),
        "w_qkv": rng.standard_normal((D, 3 * D), dtype=np.float32) * 0.03,
        "w_out": rng.standard_normal((D, D), dtype=np.float32) * 0.04,
        "b_out": np.zeros(D, dtype=np.float32),
        "scale": np.full(HEADS, DH ** -0.5, dtype=np.float32),
    }
    out = kernel(**inputs)
    print("kernel output", out.shape, out.dtype)
